# revision 21
# baseline (speedup 1.0000x reference)
"""Trainium2 Bass kernel for the Canny-edge + 1x1-conv module (v2).

Sharding: 8 cores = 4 images x 2 row-halves. Each core computes Canny on its
half (two streams: tile0 alone, tiles 1+2 merged; K=4 hysteresis), writes the
edge map to an HBM scratch tile, and streams the fused concat+1x1conv+bias+
relu output back to HBM in f16 (host upcasts to f32).

DVE throughput notes baked into the layout:
 - P0 buffers [128, n, 512]: data at even column bases -> 2x/4x DVE modes.
 - P1 buffers [128, n, 516]: data col c at 1+c, halo cols 0/513; +-1 column
   shift reads land on even bases -> 2x_1P for stencil tensor_tensor ops.
 - P2 buffers [128, n, 516]: data col c at 2+c, halo cols 1/514; aligned
   writes, PE reads all three shifts.
 - Bias is folded into the conv matmul via a ones-row in rhs; PSUM drains
   are a single-op relu (TS max) split between ACT and DVE.
"""
import numpy as np

import concourse.bass as bass
import concourse.bacc as bacc
import concourse.mybir as mybir
import concourse.tile as tile
from concourse.bass_utils import run_bass_kernel_spmd



F32 = mybir.dt.float32
F16 = mybir.dt.float16
OP = mybir.AluOpType
ACT = mybir.ActivationFunctionType

B, C, H, W = 4, 3, 512, 512
HS = 274              # shard rows: image rows [S-9, S+265)
K_HYST = 4
T_A = [0]             # stream A tile starts (within shard)
T_B = [112, 146]      # stream B tile starts
MAGIC = 8388608.0     # 2^23
C_HALF = MAGIC - 0.5  # exact in f32
T1 = 0.4142135623730951
T2 = 2.414213562373095

# conv psum drain engine pattern (per drain index, cycled): 'A'=scalar, 'D'=vector
DRAIN_PATTERN = "AD"

LAST_RESULT = None


def _canny_gen(nc, tc, pools, xs_param, mask_sb, mats, starts, seg_mask, xe, xe_rows,
               gp_heavy):
    """Emit canny for one stream of len(starts) 128-row tiles.

    starts: tile start rows within the shard.
    seg_mask: list of (seg_idx, mask_col) for segs needing row-masking.
    xe_rows: list of (tile_part_lo, tile_part_hi, xe_lo) valid spans per seg.
    gp_heavy: route more elementwise ops to gpsimd.
    """
    n = len(starts)
    NW = n * 512
    scr = pools["scratch"]
    cps = pools["cpsum"]
    dve = nc.vector
    e1 = dve   # Pool rejects TensorTensor at codegen; all elementwise on DVE

    def t(shape, dt=F16, name="t"):
        return scr.tile(shape, dt, tag=f"{name}{n}", name=f"{name}{n}")

    # ---- load x tiles ----
    xt = t([128, n, 3, 512], F32, "xt")
    for s in range(n):
        T = starts[s]
        nc.sync.dma_start(
            xt[:, s], xs_param[:, T:T + 128, :].rearrange("c h w -> h c w"))
    yield "xt"

    # ---- gray (exact f32 chain) + trunc via magic round(g - 0.5) ----
    gf = t([128, n, 512], F32, "gf")
    gP1 = t([128, n, 516], F16, "gP1")
    gP0 = t([128, n, 512], F16, "gP0")
    dve.tensor_scalar_mul(gf[:, :, :], xt[:, :, 0, :], 0.2989)
    yield
    dve.scalar_tensor_tensor(gf[:, :, :], xt[:, :, 1, :], 0.587, gf[:, :, :],
                             OP.mult, OP.add)
    yield
    dve.scalar_tensor_tensor(gf[:, :, :], xt[:, :, 2, :], 0.114, gf[:, :, :],
                             OP.mult, OP.add)
    yield
    # round_half_even(g - 0.5 + 2^23) - 2^23 == trunc(g) (g >= 0, measure-zero ties)
    dve.tensor_scalar(gP1[:, :, 1:513], gf[:, :, :], C_HALF, MAGIC, OP.add, OP.subtract)
    # reflect halo cols: g[-1] = g[1], g[512] = g[510]
    dve.tensor_copy(gP1[:, :, 0:1], gP1[:, :, 2:3])
    dve.tensor_copy(gP1[:, :, 513:514], gP1[:, :, 511:512])
    dve.tensor_copy(gP0[:, :, :], gP1[:, :, 1:513])
    yield "gray"

    # ---- sobel horizontal parts ----
    dcol = t([128, n, 512], F16, "dcol")
    hsm = t([128, n, 512], F16, "hsm")
    e1.tensor_sub(dcol[:, :, :], gP1[:, :, 2:514], gP1[:, :, 0:512])
    e1.scalar_tensor_tensor(hsm[:, :, :], gP0[:, :, :], 2.0, gP1[:, :, 0:512],
                            OP.mult, OP.add)
    e1.tensor_add(hsm[:, :, :], hsm[:, :, :], gP1[:, :, 2:514])
    yield "sobel_h"

    # ---- vertical sobel via matmul; abs on ACT; pr from psum ----
    ax = t([128, n, 512], F16, "ax")
    ay = t([128, n, 512], F16, "ay")
    pr = t([128, n, 512], F16, "pr")
    ps_gx = cps.tile([128, NW], F32, tag="cps", padded_shape=[128, 1024], name="ps_gx")
    ps_gy = cps.tile([128, NW], F32, tag="cps", padded_shape=[128, 1024], name="ps_gy")
    for s in range(n):
        nc.tensor.matmul(ps_gx[:, 512 * s:512 * s + 512], mats["tri121"][:, :],
                         dcol[:, s, :], start=True, stop=True)
    for s in range(n):
        nc.tensor.matmul(ps_gy[:, 512 * s:512 * s + 512], mats["trim101"][:, :],
                         hsm[:, s, :], start=True, stop=True)
    gxv = ps_gx.rearrange("p (n w) -> p n w", n=n)
    gyv = ps_gy.rearrange("p (n w) -> p n w", n=n)
    sx = t([128, n, 512], F16, "sx")
    nc.scalar.activation(ax[:, :, :], gxv, ACT.Abs)
    nc.scalar.activation(sx[:, :, :], gxv, ACT.Sign)
    nc.scalar.activation(ay[:, :, :], gyv, ACT.Abs)
    # pr = sign(gx) * gy  (sign only is used downstream)
    dve.scalar_tensor_tensor(pr[:, :, :], gyv, 1.0, sx[:, :, :], OP.mult, OP.mult)
    yield "sobel_v"

    # ---- mag (+ row mask on boundary segs) ----
    magP0 = t([128, n, 512], F16, "magP0")
    magP1 = t([128, n, 516], F16, "magP1")
    dve.tensor_add(magP0[:, :, :], ax[:, :, :], ay[:, :, :])
    for (s, mcol) in seg_mask:
        dve.tensor_scalar(magP0[:, s, :], magP0[:, s, :], mask_sb[:, mcol:mcol + 1],
                          None, OP.mult)
    dve.memset(magP1[:, :, 0:1], 0.0)
    dve.memset(magP1[:, :, 513:514], 0.0)
    dve.tensor_copy(magP1[:, :, 1:513], magP0[:, :, :])
    yield "mag"

    # ---- row shifts via partition-shifted SBUF->SBUF DMA ----
    maguP1 = t([128, n, 516], F16, "maguP1")
    magdP1 = t([128, n, 516], F16, "magdP1")
    zrow = pools["zrow"]
    zv = zrow[0:1, 0:n * 514].rearrange("p (n w) -> p n w", n=n)
    nc.sync.dma_start(maguP1[0:127, :, 0:514], magP1[1:128, :, 0:514])
    nc.sync.dma_start(maguP1[127:128, :, 0:514], zv)
    nc.sync.dma_start(magdP1[1:128, :, 0:514], magP1[0:127, :, 0:514])
    nc.sync.dma_start(magdP1[0:1, :, 0:514], zv)
    yield "shifts"

    # ---- direction masks ----
    U8 = mybir.dt.uint8
    c0 = t([128, n, 512], U8, "c0")
    c2 = t([128, n, 512], U8, "c2")
    c45 = t([128, n, 512], U8, "c45")
    e1.scalar_tensor_tensor(c0[:, :, :], ax[:, :, :], T1, ay[:, :, :], OP.mult, OP.is_gt)
    e1.scalar_tensor_tensor(c2[:, :, :], ax[:, :, :], T2, ay[:, :, :], OP.mult, OP.is_lt)
    dve.tensor_scalar(c45[:, :, :], pr[:, :, :], 0.0, None, OP.is_gt)
    yield "dirmask"

    # ---- NMS: per-direction neighbor max, then select, then suppress ----
    maxd0 = t([128, n, 512], F16, "maxd0")
    maxd45 = t([128, n, 512], F16, "maxd45")
    maxd90 = t([128, n, 512], F16, "maxd90")
    nmax = t([128, n, 512], F16, "nmax")
    dve.tensor_max(maxd0[:, :, :], magP1[:, :, 0:512], magP1[:, :, 2:514])
    dve.tensor_max(maxd45[:, :, :], magdP1[:, :, 2:514], maguP1[:, :, 0:512])
    dve.tensor_max(maxd90[:, :, :], maguP1[:, :, 1:513], magdP1[:, :, 1:513])
    dve.tensor_max(nmax[:, :, :], magdP1[:, :, 0:512], maguP1[:, :, 2:514])
    yield "maxd"
    dve.copy_predicated(nmax[:, :, :], c45[:, :, :], maxd45[:, :, :])
    dve.copy_predicated(nmax[:, :, :], c2[:, :, :], maxd90[:, :, :])
    dve.copy_predicated(nmax[:, :, :], c0[:, :, :], maxd0[:, :, :])
    yield "nmaxsel"
    nms = t([128, n, 512], F16, "nms")
    geq = t([128, n, 512], F16, "geq")
    e1.tensor_tensor(geq[:, :, :], magP0[:, :, :], nmax[:, :, :], OP.is_ge)
    e1.tensor_mul(nms[:, :, :], magP0[:, :, :], geq[:, :, :])
    yield "nms"

    # ---- thresholds: magnitudes (not binary) keep >0-ness equivalent ----
    weak = t([128, n, 512], F16, "weak")
    w255 = t([128, n, 512], F16, "w255")
    sA = t([128, n, 516], F16, "sA")
    sB = t([128, n, 516], F16, "sB")
    dve.tensor_scalar(sA[:, :, 2:514], nms[:, :, :], -150.0, 0.0, OP.add, OP.max)
    dve.tensor_scalar(weak[:, :, :], nms[:, :, :], -50.0, 0.0, OP.add, OP.max)
    dve.tensor_scalar(w255[:, :, :], nms[:, :, :], 50.0, 255.0, OP.is_gt, OP.mult)
    for sbuf_t in (sA, sB):
        dve.memset(sbuf_t[:, :, 1:2], 0.0)
        dve.memset(sbuf_t[:, :, 514:515], 0.0)
    yield "thresh"

    # ---- hysteresis: s' = weak * (boxsum3x3(s) > 0), K iterations ----
    edge = pools["edge_tile"]
    cur = sA
    for it in range(K_HYST):
        last = it == K_HYST - 1
        nxt = sB if (it % 2 == 0) else sA
        ps_h = cps.tile([128, NW], F32, tag="cps", padded_shape=[128, 1024],
                        name=f"ps_h{it}")
        for s in range(n):
            u = slice(512 * s, 512 * s + 512)
            nc.tensor.matmul(ps_h[:, u], mats["tri111"][:, :], cur[:, s, 1:513],
                             start=True, stop=False)
            nc.tensor.matmul(ps_h[:, u], mats["tri111"][:, :], cur[:, s, 2:514],
                             start=False, stop=False)
            nc.tensor.matmul(ps_h[:, u], mats["tri111"][:, :], cur[:, s, 3:515],
                             start=False, stop=True)
        psv = ps_h.rearrange("p (n w) -> p n w", n=n)
        if last:
            dve.scalar_tensor_tensor(edge[:, :, :], psv, 0.0, w255[:, :, :],
                                     OP.is_gt, OP.mult)
        else:
            dve.scalar_tensor_tensor(nxt[:, :, 2:514], psv, 0.0, weak[:, :, :],
                                     OP.is_gt, OP.mult)
        cur = nxt
        yield f"hyst{it}"

    yield "edge"


def build_nc():
    nc = bacc.Bacc("TRN2", target_bir_lowering=False)
    xs_param = nc.declare_dram_parameter("xs", [3, HS, W], F32, isOutput=False)
    xb_param = nc.declare_dram_parameter("xb", [8, 7, 8192], F16, isOutput=False)
    wt_param = nc.declare_dram_parameter("wt", [9, 128], F16, isOutput=False)
    mask_param = nc.declare_dram_parameter("mask", [2, 128], F32, isOutput=False)
    mats_param = nc.declare_dram_parameter("mats", [128, 5 * 128], F16, isOutput=False)
    out_param = nc.declare_dram_parameter("out", [8, 128, 8192], F16, isOutput=True)

    MAT_NAMES = ["tri121", "trim101", "shup", "shdn", "tri111"]

    with tile.TileContext(nc) as tc:
        import contextlib
        with contextlib.ExitStack() as ctx:
            const = ctx.enter_context(tc.tile_pool(name="const", bufs=1))
            scratch = ctx.enter_context(tc.tile_pool(name="scratch", bufs=1))
            epool = ctx.enter_context(tc.tile_pool(name="edges", bufs=1))
            rhs_pool = ctx.enter_context(tc.tile_pool(name="rhs", bufs=2))
            stage_pool = ctx.enter_context(tc.tile_pool(name="stage", bufs=2))
            psum_pool = ctx.enter_context(tc.tile_pool(name="psum", bufs=4, space="PSUM"))
            cpsum_pool = ctx.enter_context(tc.tile_pool(name="cpsum", bufs=2, space="PSUM"))
            xe_pool = ctx.enter_context(tc.tile_pool(name="xep", bufs=1, space="DRAM"))

            wt_sb = const.tile([9, 128], F16, tag="wt")
            mask_sb = const.tile([128, 2], F32, tag="mask")
            mats_sb = const.tile([128, 5 * 128], F16, tag="mats")
            zrow = const.tile([128, 1032], F16, tag="zrow")
            nc.vector.memset(zrow[:, :], 0.0)
            nc.scalar.dma_start(mats_sb[:, :], mats_param[:, :])
            nc.scalar.dma_start(wt_sb[:, :], wt_param[:, :])
            nc.scalar.dma_start(mask_sb[:, :], mask_param.rearrange("t p -> p t"))
            mats = {nm: mats_sb[:, 128 * i:128 * (i + 1)] for i, nm in enumerate(MAT_NAMES)}

            xe = xe_pool.tile([256, W], F16, tag="xe", name="xe")

            edgeA = epool.tile([128, 1, 512], F16, tag="edgeA", name="edgeA")
            edgeB = epool.tile([128, 2, 512], F16, tag="edgeB", name="edgeB")

            poolsA = {"scratch": scratch, "cpsum": cpsum_pool, "edge_tile": edgeA, "zrow": zrow}
            poolsB = {"scratch": scratch, "cpsum": cpsum_pool, "edge_tile": edgeB, "zrow": zrow}

            # stream A: tile0 (mask on seg0: rows S-9..S-1 invalid for half 0)
            gA = _canny_gen(nc, tc, poolsA, xs_param, mask_sb, mats, T_A,
                            seg_mask=[(0, 0)], xe=xe,
                            xe_rows=[(9, 121, 0)], gp_heavy=False)
            # stream B: tiles 1+2 (mask on seg1: rows >= 512 invalid for half 1)
            gB = _canny_gen(nc, tc, poolsB, xs_param, mask_sb, mats, T_B,
                            seg_mask=[(1, 1)], xe=xe,
                            xe_rows=[(9, 121, 112), (87, 119, 224)], gp_heavy=True)

            def drain(gen, k=10 ** 9):
                for _ in range(k):
                    if next(gen, "done") == "done":
                        return True
                return False

            drain(gA)
            # edge A -> xe[0:112]
            nc.sync.dma_start(xe[0:112, :], edgeA[9:121, 0, :])

            rhs_tiles = {}

            def load_rhs(K):
                rhs = rhs_pool.tile([9, 8192], F16, tag="rhs", name=f"rhs{K}")
                nc.sync.dma_start(rhs[0:7, :], xb_param[K])
                nc.sync.dma_start(
                    rhs[7:9, :].rearrange("g (jj hh w) -> g jj hh w", jj=4, hh=4),
                    xe[32 * K:32 * K + 32, :]
                    .rearrange("(jj g hh) w -> g jj hh w", jj=4, g=2, hh=4))
                rhs_tiles[K] = rhs

            drain_ctr = [0]

            def emit_superchunk(K, gb=None, gb_steps=0):
                rhs = rhs_tiles.pop(K)
                for half in range(2):
                    stage = stage_pool.tile([128, 4096], F16, tag="stage", name=f"st{K}_{half}")
                    for jj8 in range(8):
                        j = half * 8 + jj8
                        psum = psum_pool.tile([128, 512], F32, tag="psum", name=f"cv{K}_{j}")
                        nc.tensor.matmul(psum[:, :], wt_sb[:, :],
                                         rhs[:, 512 * j:512 * (j + 1)],
                                         start=True, stop=True)
                        o0 = 512 * jj8
                        eng = DRAIN_PATTERN[drain_ctr[0] % len(DRAIN_PATTERN)]
                        drain_ctr[0] += 1
                        if eng == "A":
                            nc.scalar.activation(stage[:, o0:o0 + 512], psum[:, :], ACT.Relu)
                        else:
                            nc.vector.tensor_scalar(stage[:, o0:o0 + 512], psum[:, :],
                                                    0.0, None, OP.max)
                        if gb is not None and jj8 == 3:
                            drain(gb, gb_steps)
                    eng = nc.scalar if half == 0 else nc.gpsimd
                    eng.dma_start(out_param[K, :, 4096 * half:4096 * (half + 1)], stage[:, :])

            load_rhs(0)
            load_rhs(1)
            for K in range(0, 3):
                if K + 2 <= 2:
                    load_rhs(K + 2)
                emit_superchunk(K, gB, 2)
            drain(gB)
            # edge B -> xe[112:224] and xe[224:256]
            nc.sync.dma_start(xe[112:224, :], edgeB[9:121, 0, :])
            nc.sync.dma_start(xe[224:256, :], edgeB[87:119, 1, :])
            load_rhs(3)
            load_rhs(4)
            for K in range(3, 8):
                if K + 2 <= 7:
                    load_rhs(K + 2)
                emit_superchunk(K)

    nc.compile()
    return nc


_NC_CACHE = None


def _host_mats():
    idx = np.arange(128)
    kk, pp = np.meshgrid(idx, idx, indexing="ij")   # [k, p]
    tri121 = np.where(kk == pp, 2.0, 0.0) + np.where(np.abs(kk - pp) == 1, 1.0, 0.0)
    trim101 = np.where(kk == pp + 1, 1.0, 0.0) - np.where(kk == pp - 1, 1.0, 0.0)
    shup = np.where(kk == pp + 1, 1.0, 0.0)
    shdn = np.where(kk == pp - 1, 1.0, 0.0)
    tri111 = np.where(np.abs(kk - pp) <= 1, 1.0, 0.0)
    m = np.stack([tri121, trim101, shup, shdn, tri111]).astype(np.float16)
    return np.ascontiguousarray(m.transpose(1, 0, 2).reshape(128, 5 * 128))


def _prep_in_maps(x, Wc, b):
    x = np.ascontiguousarray(np.asarray(x, dtype=np.float32))
    Wc = np.asarray(Wc, dtype=np.float32)
    b = np.asarray(b, dtype=np.float32)
    # rhs partition order: p = g*3 + c for x channels, 6 = ones, 7+g = edge
    wt9 = np.zeros((9, 128), np.float32)
    for g in range(2):
        wt9[g * 3:g * 3 + 3, g * 64:g * 64 + 64] = Wc[:, 0:3].T
        wt9[6, g * 64:g * 64 + 64] = b
        wt9[7 + g, g * 64:g * 64 + 64] = Wc[:, 3]
    wt9 = wt9.astype(np.float16)
    mats = _host_mats()
    in_maps = []
    for c in range(8):
        img, half = c // 2, c % 2
        S = half * 256
        rows = np.arange(S - 9, S + 265)
        rr = np.abs(rows)
        rr = np.where(rr > 511, 1022 - rr, rr)
        xs = np.ascontiguousarray(x[img][:, rr, :])
        # xb_dev[K, g*3+c, jj*2048+hh*512+w] = x[c, S + 32K+8jj+4g+hh, w]; row 6 = ones
        xh = x[img][:, S:S + 256, :].astype(np.float16)           # [3, 256, 512]
        xb = np.empty((8, 7, 8192), np.float16)
        xb[:, 0:6, :] = (
            xh.reshape(3, 8, 4, 2, 4, W).transpose(1, 3, 0, 2, 4, 5).reshape(8, 6, 8192))
        xb[:, 6, :] = 1.0
        mask = ((rows >= 0) & (rows <= 511)).astype(np.float32)
        # mask col 0: stream A seg0 (tile start 0); col 1: stream B seg1 (tile start 146)
        m2 = np.ascontiguousarray(np.stack([mask[0:128], mask[146:274]]))
        in_maps.append({"xs": xs, "xb": xb, "wt": wt9, "mask": m2, "mats": mats})
    return in_maps


def kernel(x, Wc, b):
    global _NC_CACHE, LAST_RESULT
    if _NC_CACHE is None:
        _NC_CACHE = build_nc()
    in_maps = _prep_in_maps(x, Wc, b)
    res = run_bass_kernel_spmd(_NC_CACHE, in_maps, core_ids=list(range(8)))
    LAST_RESULT = res
    out = np.empty((B, 64, H, W), np.float32)
    for c in range(8):
        img, half = c // 2, c % 2
        o = res.results[c]["out"].astype(np.float32)   # [8, 128, 8192]
        # partition = g*64+o ; free = jj*2048 + hh*512 + w ; h = 32K+8jj+4g+hh
        o = o.reshape(8, 2, 64, 4, 4, W).transpose(2, 0, 3, 1, 4, 5).reshape(64, 256, W)
        out[img, :, half * 256:(half + 1) * 256, :] = o
    return out


if __name__ == "__main__":
    d = np.load('/tmp/ref_inputs.npz')
    out = kernel(d['x'], d['Wc'], d['b'])
    ref = np.load('/tmp/ref_out.npy')
    err = np.linalg.norm(out - ref) / np.linalg.norm(ref)
    print("rel l2 err:", err, "max abs:", np.abs(out - ref).max())
